# revision 1
# baseline (speedup 1.0000x reference)
"""Trainium2 Bass kernel for RoPE + GQA causal attention (B=1, S=2048, HID=2048,
NH=16, NKV=4, HD=128), tensor-parallel over heads across 8 NeuronCores.

Sharding: core c computes q heads {2c, 2c+1} and kv head c//2, plus the
corresponding slice of the output projection (wo input-dim shard). Each core
emits a partial [S, HID] output; the host sums the 8 partials (the unshard
step for an input-dim-sharded matmul).

Per-core dataflow (all "transposed" layout, d on partitions; matmul path in
bf16 with f32 PSUM accumulation):
  xT[h,s] -> QT/KT/VT = W^T-slices @ xT          (PSUM accum over 16 h-chunks)
  RoPE(qT) = C1 * (R1 @ qT) + C2 * (R2 @ qT)     (R1/R2 are 128x128 sign-perms,
                                                  C1/C2 built from cos/sin)
  V transposed to [s,d] tiles via PE transpose
  scoresT[sk,sq] = KT^T-tile @ QT-chunk
  expT = exp(scale * scoresT + causal mask)      (ACT, bf16 out)
  attnT[d,sq] += V-tile @ expT                   (PSUM accum over sk tiles)
  den[1,sq]  += ones^T @ expT                    (PSUM accum)
  attnT = attnT / broadcast(den)                 (gpsimd bcast + DVE divide)
  partial[s,h] = attnT-slices^T @ woT            (natural layout out, f32)
"""

import os
import sys
from contextlib import ExitStack

for _p in ("/opt/trn_rl_repo", "/root/.axon_site/_ro/trn_rl_repo"):
    if os.path.isdir(_p) and _p not in sys.path:
        sys.path.append(_p)

import ml_dtypes
import numpy as np

import concourse.bass as bass
import concourse.mybir as mybir
import concourse.tile as tile
from concourse import bacc, bass_utils
from concourse.masks import make_identity

S, HID, NH, NKV, HD = 2048, 2048, 16, 4, 128
HH = HD // 2  # 64
NCORES = 8
QH_PER_CORE = NH // NCORES  # 2
SCALE = float(1.0 / np.sqrt(HD))
MASK_VAL = -30000.0

F32 = mybir.dt.float32
BF16 = mybir.dt.bfloat16
NPBF = ml_dtypes.bfloat16

SC = 512          # s-chunk width (free dim of most matmuls)
NSC = S // SC     # 4
NKC = HID // 128  # 16 contraction chunks
NST = S // 128    # 16 s-tiles


def build_graph():
    nc = bacc.Bacc(trn_type="TRN2", enable_partition_id=False)

    xT = nc.dram_tensor("xt", [HID, S], BF16, kind="ExternalInput")
    wqkvT = nc.dram_tensor("wqkvt", [HID, 4 * HD], BF16, kind="ExternalInput")
    woT = nc.dram_tensor("wot", [QH_PER_CORE * HD, HID], BF16, kind="ExternalInput")
    c1d = nc.dram_tensor("c1", [HD, S], F32, kind="ExternalInput")
    c2d = nc.dram_tensor("c2", [HD, S], F32, kind="ExternalInput")
    r1d = nc.dram_tensor("r1t", [HD, HD], BF16, kind="ExternalInput")
    r2d = nc.dram_tensor("r2t", [HD, HD], BF16, kind="ExternalInput")
    outd = nc.dram_tensor("out", [S, HID], F32, kind="ExternalOutput")

    xT_t = xT.rearrange("(ko p) s -> p ko s", p=128)       # [128, 16, 2048]
    wqkv_t = wqkvT.rearrange("(ko p) o -> p ko o", p=128)  # [128, 16, 512]
    wo_t = woT.rearrange("(g p) h -> p g h", p=128)        # [128, 2, 2048]

    with tile.TileContext(nc) as tc, ExitStack() as ctx:
        # ---- permanent pools ----------------------------------------------
        consts = ctx.enter_context(tc.tile_pool(name="consts", bufs=1))
        persist = ctx.enter_context(tc.tile_pool(name="persist", bufs=1))
        # phase-2 SBUF pools opened first so they never overlap (WAR) with
        # phase-1 pool memory
        p2s = ctx.enter_context(tc.tile_pool(name="p2s", bufs=2))
        expp = ctx.enter_context(tc.tile_pool(name="expp", bufs=4))
        outp = ctx.enter_context(tc.tile_pool(name="outp", bufs=3))

        # ---- persistent activations ---------------------------------------
        qTs = [persist.tile([128, S], BF16, tag=f"qT{h}", name=f"qT{h}")
               for h in range(QH_PER_CORE)]
        kT = persist.tile([128, S], BF16, tag="kT")
        v_sd = persist.tile([128, NST, HD], BF16, tag="v_sd")
        aoTs = [persist.tile([128, S], BF16, tag=f"aoT{h}", name=f"aoT{h}")
                for h in range(QH_PER_CORE)]

        # ---- constants -----------------------------------------------------
        ident = consts.tile([128, 128], BF16)
        make_identity(nc, ident)

        # additive causal mask for the diagonal 128x128 block of a scoresT
        # tile: keep (0) where sq >= sk i.e. col >= row, else MASK_VAL
        trimask = consts.tile([128, 128], F32)
        nc.gpsimd.memset(trimask, 0.0)
        nc.gpsimd.affine_select(
            out=trimask,
            in_=trimask,
            compare_op=mybir.AluOpType.is_ge,
            fill=MASK_VAL,
            base=0,
            pattern=[[1, 128]],       # + 1*col
            channel_multiplier=-1,    # - row
        )

        ones_col = consts.tile([128, 1], BF16)
        nc.vector.memset(ones_col, 1.0)

        r1_sb = consts.tile([128, 128], BF16)
        r2_sb = consts.tile([128, 128], BF16)
        wo_sb = consts.tile([128, QH_PER_CORE, HID], BF16)

        # ================= phase 1: projections + RoPE =====================
        with tc.tile_pool(name="p1c", bufs=1) as p1c, \
             tc.tile_pool(name="p1s", bufs=2) as p1s, \
             tc.tile_pool(name="ps_proj", bufs=3, space="PSUM") as ps_proj, \
             tc.tile_pool(name="ps_rope", bufs=2, space="PSUM") as ps_rope, \
             tc.tile_pool(name="ps_vt", bufs=1, space="PSUM") as ps_vt:

            wqkv_sb = p1c.tile([128, NKC, 512], BF16)
            xt0 = p1s.tile([128, NKC, SC], BF16, tag="xt", name="xt0")
            # first compute chunk's inputs lead the DMA queues; kc-interleaved
            # and split across HW (sync) + SW (gpsimd) DGE queues
            for kc in range(NKC):
                eng = nc.sync if kc % 2 == 0 else nc.gpsimd
                eng.dma_start(wqkv_sb[:, kc, :], wqkv_t[:, kc, :])
                eng2 = nc.gpsimd if kc % 2 == 0 else nc.sync
                eng2.dma_start(xt0[:, kc, :], xT_t[:, kc, 0:SC])
            c1_sb = p1c.tile([128, S], F32)
            c2_sb = p1c.tile([128, S], F32)
            nc.sync.dma_start(c1_sb, c1d[:, :])
            nc.sync.dma_start(c2_sb, c2d[:, :])
            nc.sync.dma_start(r1_sb, r1d[:, :])
            nc.sync.dma_start(r2_sb, r2d[:, :])

            def rope_into(dst_chunk, psum_raw, j):
                """dst = C1*(R1@raw) + C2*(R2@raw), raw read from PSUM."""
                raw = p1s.tile([128, SC], BF16, tag="rope_raw", name="rope_raw")
                nc.scalar.copy(out=raw, in_=psum_raw)
                ps_u = ps_rope.tile([128, SC], F32, tag="rope_uv", name="ps_u")
                ps_w = ps_rope.tile([128, SC], F32, tag="rope_uv", name="ps_w")
                nc.tensor.matmul(ps_u, r1_sb, raw, start=True, stop=True)
                nc.tensor.matmul(ps_w, r2_sb, raw, start=True, stop=True)
                csl = slice(j * SC, (j + 1) * SC)
                t1 = p1s.tile([128, SC], F32, tag="rope_t1", name="t1")
                t2 = p1s.tile([128, SC], F32, tag="rope_t2", name="t2")
                nc.vector.tensor_mul(out=t1, in0=ps_u, in1=c1_sb[:, csl])
                nc.vector.tensor_mul(out=t2, in0=ps_w, in1=c2_sb[:, csl])
                nc.vector.tensor_add(out=dst_chunk, in0=t1, in1=t2)

            for j in range(NSC):
                csl = slice(j * SC, (j + 1) * SC)
                if j == 0:
                    xt = xt0
                else:
                    xt = p1s.tile([128, NKC, SC], BF16, tag="xt", name="xt")
                    for kc in range(NKC):
                        nc.sync.dma_start(xt[:, kc, :], xT_t[:, kc, csl])

                for t in range(4):  # q0, q1, k, v (columns of wqkv)
                    osl = slice(t * 128, (t + 1) * 128)
                    ps_p = ps_proj.tile([128, SC], F32, tag="mm", name="ps_p")
                    for kc in range(NKC):
                        nc.tensor.matmul(
                            ps_p,
                            wqkv_sb[:, kc, osl],
                            xt[:, kc, :],
                            start=(kc == 0),
                            stop=(kc == NKC - 1),
                        )
                    if t < 2:
                        rope_into(qTs[t][:, csl], ps_p, j)
                    elif t == 2:
                        rope_into(kT[:, csl], ps_p, j)
                    else:
                        # V^T chunk -> bf16 -> PE-transpose into [s,d] tiles
                        vt_raw = p1s.tile([128, SC], BF16, tag="vt_raw",
                                          name="vt_raw")
                        nc.scalar.copy(out=vt_raw, in_=ps_p)
                        for b in range(SC // 128):
                            st = j * (SC // 128) + b
                            ps_t = ps_vt.tile([128, 128], BF16, tag="vtp",
                                              name="ps_t")
                            nc.tensor.transpose(
                                ps_t, vt_raw[:, b * 128:(b + 1) * 128], ident
                            )
                            nc.scalar.copy(out=v_sd[:, st, :], in_=ps_t)

            # wo load at the tail of phase 1 (needed from phase 3 on)
            for g in range(QH_PER_CORE):
                for hc in range(NSC):
                    nc.sync.dma_start(
                        wo_sb[:, g, hc * SC:(hc + 1) * SC],
                        wo_t[:, g, hc * SC:(hc + 1) * SC],
                    )

        # ============ phase 2+3: attention + output projection =============
        # out-projection of chunk j is emitted after attention of chunk j+1
        # so the PE never waits on the softmax-normalization chain.
        with tc.tile_pool(name="ps_mm", bufs=3, space="PSUM") as ps_mm, \
             tc.tile_pool(name="ps_acc", bufs=2, space="PSUM") as ps_acc, \
             tc.tile_pool(name="ps_den", bufs=1, space="PSUM") as ps_den, \
             tc.tile_pool(name="ps_outc", bufs=2, space="PSUM") as ps_outc:

            def attention_chunk(j):
                csl = slice(j * SC, (j + 1) * SC)
                nk = 4 * j + 4  # causal: sk tiles 0..4j+3
                qcs = [qTs[h][:, csl] for h in range(QH_PER_CORE)]
                ps_os = [ps_acc.tile([128, SC], F32, tag="attn",
                                     name=f"ps_o{h}")
                         for h in range(QH_PER_CORE)]
                ps_dall = ps_den.tile([64, SC], F32, tag="den", name="ps_dall")
                ps_ds = [ps_dall[32 * h:32 * h + 1, :]
                         for h in range(QH_PER_CORE)]
                # software-pipelined: scores/exp for step k are emitted
                # before attnV/den of step k-1, so the PE never waits on the
                # ACT exp latency
                pend = None  # (k, vsl, e_tiles)
                for k in range(nk):
                    m = k - 4 * j
                    # diagonal tiles (m>=0) only touch cols >= 128m
                    v0 = max(m, 0) * 128
                    vsl = slice(v0, SC)
                    kc_t = kT[:, k * 128:(k + 1) * 128]
                    es = []
                    for h in range(QH_PER_CORE):
                        ps_s = ps_mm.tile([128, SC], F32, tag="mm",
                                          name="ps_s")
                        nc.tensor.matmul(
                            ps_s[:, vsl], kc_t, qcs[h][:, vsl],
                            start=True, stop=True,
                        )
                        e = expp.tile([128, SC], BF16, tag="exp", name="e")
                        if m >= 0:
                            dsl = slice(m * 128, (m + 1) * 128)
                            nc.vector.tensor_add(
                                out=ps_s[:, dsl], in0=ps_s[:, dsl],
                                in1=trimask,
                            )
                        nc.scalar.activation(
                            out=e[:, vsl], in_=ps_s[:, vsl],
                            func=mybir.ActivationFunctionType.Exp,
                            scale=SCALE,
                        )
                        es.append(e)
                    if pend is not None:
                        pk, pvsl, pes = pend
                        for h in range(QH_PER_CORE):
                            nc.tensor.matmul(
                                ps_os[h][:, pvsl], v_sd[:, pk, :],
                                pes[h][:, pvsl],
                                start=(pk == 0), stop=False,
                            )
                            nc.tensor.matmul(
                                ps_ds[h][:, pvsl], ones_col, pes[h][:, pvsl],
                                start=(pk == 0), stop=False,
                            )
                    pend = (k, vsl, es)
                pk, pvsl, pes = pend
                for h in range(QH_PER_CORE):
                    nc.tensor.matmul(
                        ps_os[h][:, pvsl], v_sd[:, pk, :], pes[h][:, pvsl],
                        start=(pk == 0), stop=True,
                    )
                    nc.tensor.matmul(
                        ps_ds[h][:, pvsl], ones_col, pes[h][:, pvsl],
                        start=(pk == 0), stop=True,
                    )
                for h in range(QH_PER_CORE):
                    # free the PSUM banks right away: raw attn -> SBUF, den
                    # via ACT ln; the normalization itself happens in-place
                    # later (out-proj lags a chunk, so there is slack)
                    nc.vector.tensor_copy(out=aoTs[h][:, csl], in_=ps_os[h])
                    lnd = p2s.tile([1, SC], F32, tag="lnd", name=f"lnd{h}")
                    nc.scalar.activation(
                        out=lnd, in_=ps_ds[h],
                        func=mybir.ActivationFunctionType.Ln,
                    )
                    recip = p2s.tile([1, SC], F32, tag="recip",
                                     name=f"recip{h}")
                    nc.scalar.activation(
                        out=recip, in_=lnd,
                        func=mybir.ActivationFunctionType.Exp,
                        scale=-1.0,
                    )
                    db = p2s.tile([128, SC], F32, tag="den_b", name=f"db{h}")
                    nc.gpsimd.partition_broadcast(db, recip[:1, :])
                    nc.vector.tensor_mul(out=aoTs[h][:, csl],
                                         in0=aoTs[h][:, csl], in1=db)

            def outproj_chunk(j):
                for b in range(SC // 128):
                    st = j * (SC // 128) + b
                    ssl = slice(st * 128, (st + 1) * 128)
                    for hc in range(NSC):
                        hsl = slice(hc * SC, (hc + 1) * SC)
                        ps_c = ps_outc.tile([128, SC], F32, tag="oc",
                                            name="ps_c")
                        for h in range(QH_PER_CORE):
                            nc.tensor.matmul(
                                ps_c,
                                aoTs[h][:, ssl],
                                wo_sb[:, h, hsl],
                                start=(h == 0),
                                stop=(h == QH_PER_CORE - 1),
                            )
                        ob = outp.tile([128, SC], F32, tag="outb", name="ob")
                        if j == NSC - 1 and hc % 2 == 1:
                            nc.scalar.copy(out=ob, in_=ps_c)
                        else:
                            nc.vector.tensor_copy(out=ob, in_=ps_c)
                        nc.sync.dma_start(outd[ssl, hsl], ob)

            attention_chunk(0)
            for j in range(1, NSC):
                attention_chunk(j)
                outproj_chunk(j - 1)
            outproj_chunk(NSC - 1)

    nc.finalize()
    return nc


def shard_inputs(x, cos, sin, wq, wk, wv, wo):
    x = np.asarray(x, np.float32).reshape(S, HID)
    cos = np.asarray(cos, np.float32)
    sin = np.asarray(sin, np.float32)
    wq = np.asarray(wq, np.float32)
    wk = np.asarray(wk, np.float32)
    wv = np.asarray(wv, np.float32)
    wo = np.asarray(wo, np.float32)

    xT = np.ascontiguousarray(x.T).astype(NPBF)

    cos_h, sin_h = cos[:, :HH].T, sin[:, :HH].T       # [64, S]
    c1 = np.ascontiguousarray(np.concatenate([cos_h, -sin_h], axis=0))
    c2 = np.ascontiguousarray(np.concatenate([sin_h, cos_h], axis=0))

    r1 = np.zeros((HD, HD), np.float32)
    for i in range(HH // 2):
        r1[2 * i, 2 * i + 1] = -1.0
        r1[2 * i + 1, 2 * i] = 1.0
    r1[HH:, :] = r1[:HH, :]
    r2 = np.zeros((HD, HD), np.float32)
    for d in range(HH):
        r2[d, d + HH] = 1.0
        r2[d + HH, d + HH] = 1.0
    r1t = np.ascontiguousarray(r1.T).astype(NPBF)  # lhsT for out = R1 @ rhs
    r2t = np.ascontiguousarray(r2.T).astype(NPBF)

    in_maps = []
    for c in range(NCORES):
        h0 = QH_PER_CORE * c
        kvh = h0 * NKV // NH
        wq_c = wq[h0 * HD:(h0 + QH_PER_CORE) * HD, :]    # [256, HID]
        wk_c = wk[kvh * HD:(kvh + 1) * HD, :]            # [128, HID]
        wv_c = wv[kvh * HD:(kvh + 1) * HD, :]
        wqkvT_c = np.ascontiguousarray(
            np.concatenate([wq_c, wk_c, wv_c], axis=0).T).astype(NPBF)
        woT_c = np.ascontiguousarray(
            wo[:, h0 * HD:(h0 + QH_PER_CORE) * HD].T).astype(NPBF)
        in_maps.append({
            "xt": xT,
            "wqkvt": wqkvT_c,
            "wot": woT_c,
            "c1": c1,
            "c2": c2,
            "r1t": r1t,
            "r2t": r2t,
        })
    return in_maps


_CACHED_NC = None


def kernel(x, cos, sin, wq, wk, wv, wo, _trace=False, _tmpdir=None):
    global _CACHED_NC
    in_maps = shard_inputs(x, cos, sin, wq, wk, wv, wo)
    if _CACHED_NC is None:
        _CACHED_NC = build_graph()
    nc = _CACHED_NC
    res = bass_utils.run_bass_kernel_spmd(
        nc, in_maps, core_ids=list(range(NCORES)),
        trace=_trace, tmpdir=_tmpdir,
    )
    total = np.zeros((S, HID), np.float32)
    for r in res.results:
        total += r["out"]
    out = total.reshape(1, S, HID)
    if _trace:
        return out, res
    return out



# revision 2
# speedup vs baseline: 1.1207x; 1.1207x over previous
"""Trainium2 Bass kernel for RoPE + GQA causal attention (B=1, S=2048, HID=2048,
NH=16, NKV=4, HD=128), tensor-parallel over heads across 8 NeuronCores.

Sharding: core c computes q heads {2c, 2c+1} and kv head c//2, plus the
corresponding slice of the output projection (wo input-dim shard). Each core
emits a partial [S, HID] output in bf16; the host sums the 8 partials.

Per-core dataflow (transposed layout, d on partitions; bf16 matmuls, f32 PSUM):
  phase 1 (kc-major projections, software-pipelined RoPE/V-transpose):
    xT[h,s] -> QT/KT/VT = W^T-slices @ xT     (4 PSUM accumulators, kc outer)
    RoPE(qT) = C1 * (R1 @ qT) + C2 * (R2 @ qT)
    V transposed to [s,d] tiles via PE transpose
  phase 2 (attention + lagged output projection):
    scoresT[sk,sq] = KT^T-tile @ QT-chunk     -> exp on ACT (only ACT func)
    causal mask applied as 0/1 multiply on the exp tile (DVE)
    attnT[d,sq] += V-tile @ expT              (PSUM accum over sk tiles)
    den[1,sq]  += ones^T @ expT               (pairs col-packed, partitions
                                               0/32 -> concurrent col groups)
    recip = reciprocal_approx_fast(den)       (single DVE op, no ACT tables)
    db = ones_row^T @ recip                   (PE partition-broadcast)
    aoT = attnT * db                          (DVE, writes bf16)
    partial[s,h] = aoT-slices^T @ woT         (batched [128,HID] bf16 out DMA)
"""

import os
import sys
from contextlib import ExitStack

for _p in ("/opt/trn_rl_repo", "/root/.axon_site/_ro/trn_rl_repo"):
    if os.path.isdir(_p) and _p not in sys.path:
        sys.path.append(_p)

import ml_dtypes
import numpy as np

import concourse.bass as bass
import concourse.mybir as mybir
import concourse.tile as tile
from concourse import bacc, bass_utils
from concourse.masks import make_identity

S, HID, NH, NKV, HD = 2048, 2048, 16, 4, 128
HH = HD // 2  # 64
NCORES = 8
QH_PER_CORE = NH // NCORES  # 2
SCALE = float(1.0 / np.sqrt(HD))

F32 = mybir.dt.float32
BF16 = mybir.dt.bfloat16
NPBF = ml_dtypes.bfloat16

SC = 512          # s-chunk width (free dim of most matmuls)
NSC = S // SC     # 4
NKC = HID // 128  # 16 contraction chunks
NST = S // 128    # 16 s-tiles


def build_graph():
    nc = bacc.Bacc(trn_type="TRN2", enable_partition_id=False)

    xT = nc.dram_tensor("xt", [HID, S], BF16, kind="ExternalInput")
    wqkvT = nc.dram_tensor("wqkvt", [HID, 4 * HD], BF16, kind="ExternalInput")
    woT = nc.dram_tensor("wot", [QH_PER_CORE * HD, HID], BF16, kind="ExternalInput")
    c1d = nc.dram_tensor("c1", [HD, S], BF16, kind="ExternalInput")
    c2d = nc.dram_tensor("c2", [HD, S], BF16, kind="ExternalInput")
    r1d = nc.dram_tensor("r1t", [HD, HD], BF16, kind="ExternalInput")
    r2d = nc.dram_tensor("r2t", [HD, HD], BF16, kind="ExternalInput")
    outd = nc.dram_tensor("out", [S, HID], BF16, kind="ExternalOutput")

    xT_t = xT.rearrange("(ko p) s -> p ko s", p=128)       # [128, 16, 2048]
    wqkv_t = wqkvT.rearrange("(ko p) o -> p ko o", p=128)  # [128, 16, 512]
    wo_t = woT.rearrange("(g p) h -> p g h", p=128)        # [128, 2, 2048]

    with tile.TileContext(nc) as tc, ExitStack() as ctx:
        # ---- permanent pools ----------------------------------------------
        consts = ctx.enter_context(tc.tile_pool(name="consts", bufs=1))
        persist = ctx.enter_context(tc.tile_pool(name="persist", bufs=1))
        # phase-2 SBUF pools opened before phase-1 pools so their memory
        # never overlaps (no WAR serialization at the phase boundary)
        p2s = ctx.enter_context(tc.tile_pool(name="p2s", bufs=2))
        expp = ctx.enter_context(tc.tile_pool(name="expp", bufs=4))
        outp = ctx.enter_context(tc.tile_pool(name="outp", bufs=3))

        # ---- persistent activations ---------------------------------------
        qTs = [persist.tile([128, S], BF16, tag=f"qT{h}", name=f"qT{h}")
               for h in range(QH_PER_CORE)]
        kT = persist.tile([128, S], BF16, tag="kT")
        v_sd = persist.tile([128, NST, HD], BF16, tag="v_sd")
        aoTs = [persist.tile([128, S], BF16, tag=f"aoT{h}", name=f"aoT{h}")
                for h in range(QH_PER_CORE)]
        xsb = persist.tile([128, NKC, S], BF16, tag="xsb")

        # ---- constants -----------------------------------------------------
        ident = consts.tile([128, 128], BF16)
        make_identity(nc, ident)

        # 0/1 causal mask for the diagonal 128x128 block of an expT tile:
        # keep (1) where sq >= sk i.e. col >= row, else 0
        mask01 = consts.tile([128, 128], BF16)
        nc.gpsimd.memset(mask01, 1.0)
        nc.gpsimd.affine_select(
            out=mask01,
            in_=mask01,
            compare_op=mybir.AluOpType.is_ge,
            fill=0.0,
            base=0,
            pattern=[[1, 128]],       # + 1*col
            channel_multiplier=-1,    # - row
        )

        ones_col = consts.tile([128, 1], BF16)
        nc.vector.memset(ones_col, 1.0)
        # f32 ones rows at partitions 0 and 32 (lhsT of the den broadcast)
        ones_f32 = consts.tile([64, 128], F32)
        nc.vector.memset(ones_f32, 1.0)

        r1_sb = consts.tile([128, 128], BF16)
        r2_sb = consts.tile([128, 128], BF16)
        wo_sb = consts.tile([128, QH_PER_CORE, HID], BF16)
        c1_sb = consts.tile([128, S], BF16)
        c2_sb = consts.tile([128, S], BF16)

        # ================= phase 1: projections + RoPE =====================
        with tc.tile_pool(name="p1c", bufs=1) as p1c, \
             tc.tile_pool(name="p1s", bufs=8) as p1s, \
             tc.tile_pool(name="p1t", bufs=2) as p1t, \
             tc.tile_pool(name="ps_proj", bufs=5, space="PSUM") as ps_proj, \
             tc.tile_pool(name="ps_rope", bufs=1, space="PSUM") as ps_rope, \
             tc.tile_pool(name="ps_vt", bufs=1, space="PSUM") as ps_vt:

            wqkv_sb = p1c.tile([128, NKC, 512], BF16)
            # weights on the sync HW queue, x chunks interleaved on
            # scalar + sync HW queues in consumption (j, kc) order
            for kc in range(NKC):
                nc.sync.dma_start(wqkv_sb[:, kc, :], wqkv_t[:, kc, :])
            for j in range(NSC):
                csl = slice(j * SC, (j + 1) * SC)
                for kc in range(NKC):
                    eng = nc.scalar if kc % 2 == 0 else nc.sync
                    eng.dma_start(xsb[:, kc, csl], xT_t[:, kc, csl])
            # constants on the gpsimd (SW) queue - off the critical path
            nc.gpsimd.dma_start(c1_sb, c1d[:, :])
            nc.gpsimd.dma_start(c2_sb, c2d[:, :])
            nc.gpsimd.dma_start(r1_sb, r1d[:, :])
            nc.gpsimd.dma_start(r2_sb, r2d[:, :])
            for g in range(QH_PER_CORE):
                nc.gpsimd.dma_start(wo_sb[:, g, :], wo_t[:, g, :])

            def rope_into(dst_chunk, raw, j):
                """dst = C1*(R1@raw) + C2*(R2@raw)."""
                ps_u = ps_rope.tile([128, SC], F32, tag="rope_u", name="ps_u")
                ps_w = ps_rope.tile([128, SC], F32, tag="rope_w", name="ps_w")
                nc.tensor.matmul(ps_u, r1_sb, raw, start=True, stop=True)
                nc.tensor.matmul(ps_w, r2_sb, raw, start=True, stop=True)
                csl = slice(j * SC, (j + 1) * SC)
                t1 = p1t.tile([128, SC], F32, tag="rope_t1", name="t1")
                t2 = p1t.tile([128, SC], F32, tag="rope_t2", name="t2")
                nc.vector.tensor_mul(out=t1, in0=ps_u, in1=c1_sb[:, csl])
                nc.vector.tensor_mul(out=t2, in0=ps_w, in1=c2_sb[:, csl])
                nc.vector.tensor_add(out=dst_chunk, in0=t1, in1=t2)

            def consume(j, raws):
                csl = slice(j * SC, (j + 1) * SC)
                for t in range(QH_PER_CORE):
                    rope_into(qTs[t][:, csl], raws[t], j)
                rope_into(kT[:, csl], raws[2], j)
                for b in range(SC // 128):
                    st = j * (SC // 128) + b
                    ps_t = ps_vt.tile([128, 128], BF16, tag="vtp", name="ps_t")
                    nc.tensor.transpose(
                        ps_t, raws[3][:, b * 128:(b + 1) * 128], ident
                    )
                    nc.vector.tensor_copy(out=v_sd[:, st, :], in_=ps_t)

            pend = None
            for j in range(NSC):
                csl = slice(j * SC, (j + 1) * SC)
                ps_ps = [ps_proj.tile([128, SC], F32, tag="mm", name=f"pp{t}")
                         for t in range(4)]
                for kc in range(NKC):
                    for t in range(4):  # q0, q1, k, v (columns of wqkv)
                        nc.tensor.matmul(
                            ps_ps[t],
                            wqkv_sb[:, kc, t * 128:(t + 1) * 128],
                            xsb[:, kc, csl],
                            start=(kc == 0),
                            stop=(kc == NKC - 1),
                        )
                raws = [p1s.tile([128, SC], BF16, tag="raw", name=f"raw{t}")
                        for t in range(4)]
                for t in range(4):
                    nc.vector.tensor_copy(out=raws[t], in_=ps_ps[t])
                if pend is not None:
                    consume(*pend)
                pend = (j, raws)
            consume(*pend)

        # ============ phase 2: attention + output projection ===============
        with tc.tile_pool(name="ps_mm", bufs=3, space="PSUM") as ps_mm, \
             tc.tile_pool(name="ps_acc", bufs=2, space="PSUM") as ps_acc, \
             tc.tile_pool(name="ps_den", bufs=1, space="PSUM") as ps_den, \
             tc.tile_pool(name="ps_outc", bufs=2, space="PSUM") as ps_outc:

            chunk_state = {}

            def attention_chunk(j):
                csl = slice(j * SC, (j + 1) * SC)
                nk = 4 * j + 4  # causal: sk tiles 0..4j+3
                qcs = [qTs[h][:, csl] for h in range(QH_PER_CORE)]
                ps_os = [ps_acc.tile([128, SC], F32, tag="attn",
                                     name=f"ps_o{h}")
                         for h in range(QH_PER_CORE)]
                ps_dall = ps_den.tile([64, SC], F32, tag="den", name="ps_dall")
                ps_ds = [ps_dall[32 * h:32 * h + 1, :]
                         for h in range(QH_PER_CORE)]
                # software-pipelined: scores/exp for step k are emitted
                # before attnV/den of step k-1, so the PE never waits on the
                # ACT exp latency
                pend = None  # (k, vsl, e_tiles)

                def flush(pk, pvsl, pes, last):
                    # AV pair first, then the den pair adjacent (the two den
                    # matmuls hit col groups 0/1 and run concurrently);
                    # on the last step den goes first so the recip chain
                    # starts while the final AV pair still streams
                    av = [(h, 'av') for h in range(QH_PER_CORE)]
                    dn = [(h, 'dn') for h in range(QH_PER_CORE)]
                    order = dn + av if last else av + dn
                    for h, kind in order:
                        if kind == 'av':
                            nc.tensor.matmul(
                                ps_os[h][:, pvsl], v_sd[:, pk, :],
                                pes[h][:, pvsl],
                                start=(pk == 0), stop=last,
                            )
                        else:
                            nc.tensor.matmul(
                                ps_ds[h][:, pvsl], ones_col, pes[h][:, pvsl],
                                start=(pk == 0), stop=last,
                            )

                for k in range(nk):
                    m = k - 4 * j
                    # diagonal tiles (m>=0) only touch cols >= 128m
                    v0 = max(m, 0) * 128
                    vsl = slice(v0, SC)
                    kc_t = kT[:, k * 128:(k + 1) * 128]
                    es = []
                    for h in range(QH_PER_CORE):
                        ps_s = ps_mm.tile([128, SC], F32, tag="mm",
                                          name="ps_s")
                        nc.tensor.matmul(
                            ps_s[:, vsl], kc_t, qcs[h][:, vsl],
                            start=True, stop=True,
                        )
                        e = expp.tile([128, SC], BF16, tag="exp", name="e")
                        nc.scalar.activation(
                            out=e[:, vsl], in_=ps_s[:, vsl],
                            func=mybir.ActivationFunctionType.Exp,
                            scale=SCALE,
                        )
                        if m >= 0:
                            dsl = slice(m * 128, (m + 1) * 128)
                            nc.vector.tensor_mul(
                                out=e[:, dsl], in0=e[:, dsl], in1=mask01,
                            )
                        es.append(e)
                    if pend is not None:
                        flush(*pend, last=False)
                    pend = (k, vsl, es)
                flush(*pend, last=True)
                chunk_state[j] = (ps_os, ps_dall)

            def norm_chunk(j):
                """den -> recip (DVE) -> PE broadcast -> normalize into aoT."""
                csl = slice(j * SC, (j + 1) * SC)
                ps_os, ps_dall = chunk_state.pop(j)
                recip = p2s.tile([64, SC], F32, tag="recip", name="recip")
                nc.vector.reciprocal_approx_fast(out=recip, in_=ps_dall)
                for h in range(QH_PER_CORE):
                    db_ps = ps_mm.tile([128, SC], F32, tag="mm",
                                       name=f"db{h}")
                    nc.tensor.matmul(
                        db_ps,
                        ones_f32[32 * h:32 * h + 1, :],
                        recip[32 * h:32 * h + 1, :],
                        start=True, stop=True,
                    )
                    db_s = p2s.tile([128, SC], F32, tag="db_s",
                                    name=f"dbs{h}")
                    nc.vector.tensor_copy(out=db_s, in_=db_ps)
                    nc.vector.tensor_mul(out=aoTs[h][:, csl],
                                         in0=ps_os[h], in1=db_s)

            def outproj_st(j, b):
                """one 128-row block of the output projection of chunk j."""
                st = j * (SC // 128) + b
                ssl = slice(st * 128, (st + 1) * 128)
                ob = outp.tile([128, HID], BF16, tag="outb", name="ob")
                for hc in range(NSC):
                    hsl = slice(hc * SC, (hc + 1) * SC)
                    ps_c = ps_outc.tile([128, SC], F32, tag="oc",
                                        name="ps_c")
                    for h in range(QH_PER_CORE):
                        nc.tensor.matmul(
                            ps_c,
                            aoTs[h][:, ssl],
                            wo_sb[:, h, hsl],
                            start=(h == 0),
                            stop=(h == QH_PER_CORE - 1),
                        )
                    nc.vector.tensor_copy(out=ob[:, hsl], in_=ps_c)
                nc.sync.dma_start(outd[ssl, :], ob)

            attention_chunk(0)
            norm_chunk(0)
            for j in range(1, NSC):
                attention_chunk(j)
                outproj_st(j - 1, 0)
                norm_chunk(j)
                for b in range(1, SC // 128):
                    outproj_st(j - 1, b)
            for b in range(SC // 128):
                outproj_st(NSC - 1, b)

    nc.finalize()
    return nc


def shard_inputs(x, cos, sin, wq, wk, wv, wo):
    x = np.asarray(x, np.float32).reshape(S, HID)
    cos = np.asarray(cos, np.float32)
    sin = np.asarray(sin, np.float32)
    wq = np.asarray(wq, np.float32)
    wk = np.asarray(wk, np.float32)
    wv = np.asarray(wv, np.float32)
    wo = np.asarray(wo, np.float32)

    xT = np.ascontiguousarray(x.T).astype(NPBF)

    cos_h, sin_h = cos[:, :HH].T, sin[:, :HH].T       # [64, S]
    c1 = np.ascontiguousarray(
        np.concatenate([cos_h, -sin_h], axis=0)).astype(NPBF)
    c2 = np.ascontiguousarray(
        np.concatenate([sin_h, cos_h], axis=0)).astype(NPBF)

    r1 = np.zeros((HD, HD), np.float32)
    for i in range(HH // 2):
        r1[2 * i, 2 * i + 1] = -1.0
        r1[2 * i + 1, 2 * i] = 1.0
    r1[HH:, :] = r1[:HH, :]
    r2 = np.zeros((HD, HD), np.float32)
    for d in range(HH):
        r2[d, d + HH] = 1.0
        r2[d + HH, d + HH] = 1.0
    r1t = np.ascontiguousarray(r1.T).astype(NPBF)  # lhsT for out = R1 @ rhs
    r2t = np.ascontiguousarray(r2.T).astype(NPBF)

    in_maps = []
    for c in range(NCORES):
        h0 = QH_PER_CORE * c
        kvh = h0 * NKV // NH
        wq_c = wq[h0 * HD:(h0 + QH_PER_CORE) * HD, :]    # [256, HID]
        wk_c = wk[kvh * HD:(kvh + 1) * HD, :]            # [128, HID]
        wv_c = wv[kvh * HD:(kvh + 1) * HD, :]
        wqkvT_c = np.ascontiguousarray(
            np.concatenate([wq_c, wk_c, wv_c], axis=0).T).astype(NPBF)
        woT_c = np.ascontiguousarray(
            wo[:, h0 * HD:(h0 + QH_PER_CORE) * HD].T).astype(NPBF)
        in_maps.append({
            "xt": xT,
            "wqkvt": wqkvT_c,
            "wot": woT_c,
            "c1": c1,
            "c2": c2,
            "r1t": r1t,
            "r2t": r2t,
        })
    return in_maps


_CACHED_NC = None


def kernel(x, cos, sin, wq, wk, wv, wo, _trace=False, _tmpdir=None):
    global _CACHED_NC
    in_maps = shard_inputs(x, cos, sin, wq, wk, wv, wo)
    if _CACHED_NC is None:
        _CACHED_NC = build_graph()
    nc = _CACHED_NC
    res = bass_utils.run_bass_kernel_spmd(
        nc, in_maps, core_ids=list(range(NCORES)),
        trace=_trace, tmpdir=_tmpdir,
    )
    total = np.zeros((S, HID), np.float32)
    for r in res.results:
        total += np.asarray(r["out"], dtype=np.float32)
    out = total.reshape(1, S, HID)
    if _trace:
        return out, res
    return out


# revision 7
# speedup vs baseline: 1.1491x; 1.0254x over previous
"""Trainium2 Bass kernel for RoPE + GQA causal attention (B=1, S=2048, HID=2048,
NH=16, NKV=4, HD=128), tensor-parallel over heads across 8 NeuronCores.

Sharding: core c computes q heads {2c, 2c+1} and kv head c//2, plus the
corresponding slice of the output projection (wo input-dim shard). Each core
emits a partial [S, HID] output in bf16; the host sums the 8 partials.

Per-core dataflow (transposed layout, d on partitions; bf16 matmuls, f32 PSUM):
  phase 1 (kc-major projections, software-pipelined RoPE/V-transpose):
    xT[h,s] -> QT/KT/VT = W^T-slices @ xT     (4 PSUM accumulators, kc outer)
    RoPE(qT) = C1 * (R1 @ qT) + C2 * (R2 @ qT)
    V transposed to [s,d] tiles via PE transpose
  phase 2 (attention + lagged output projection):
    scoresT[sk,sq] = KT^T-tile @ QT-chunk     -> exp on ACT (only ACT func)
    causal mask applied as 0/1 multiply on the exp tile (DVE)
    attnT[d,sq] += V-tile @ expT              (PSUM accum over sk tiles)
    den[1,sq]  += ones^T @ expT               (pairs col-packed, partitions
                                               0/32 -> concurrent col groups)
    recip = reciprocal_approx_fast(den)       (single DVE op, no ACT tables)
    db = ones_row^T @ recip                   (PE partition-broadcast)
    aoT = attnT * db                          (DVE, writes bf16)
    partial[s,h] = aoT-slices^T @ woT         (batched [128,HID] bf16 out DMA)
"""

import os
import sys
from contextlib import ExitStack

for _p in ("/opt/trn_rl_repo", "/root/.axon_site/_ro/trn_rl_repo"):
    if os.path.isdir(_p) and _p not in sys.path:
        sys.path.append(_p)

import ml_dtypes
import numpy as np

import concourse.bass as bass
import concourse.mybir as mybir
import concourse.tile as tile
from concourse import bacc, bass_utils
from concourse.masks import make_identity

S, HID, NH, NKV, HD = 2048, 2048, 16, 4, 128
HH = HD // 2  # 64
NCORES = 8
QH_PER_CORE = NH // NCORES  # 2
SCALE = float(1.0 / np.sqrt(HD))

F32 = mybir.dt.float32
BF16 = mybir.dt.bfloat16
NPBF = ml_dtypes.bfloat16

SC = 512          # s-chunk width (free dim of most matmuls)
NSC = S // SC     # 4
NKC = HID // 128  # 16 contraction chunks
NST = S // 128    # 16 s-tiles


def build_graph():
    nc = bacc.Bacc(trn_type="TRN2", enable_partition_id=False)

    xT = nc.dram_tensor("xt", [HID, S], BF16, kind="ExternalInput")
    wqkvT = nc.dram_tensor("wqkvt", [HID, 4 * HD], BF16, kind="ExternalInput")
    woT = nc.dram_tensor("wot", [QH_PER_CORE * HD, HID], BF16, kind="ExternalInput")
    c1d = nc.dram_tensor("c1", [HD, S], BF16, kind="ExternalInput")
    c2d = nc.dram_tensor("c2", [HD, S], BF16, kind="ExternalInput")
    r1d = nc.dram_tensor("r1t", [HD, HD], BF16, kind="ExternalInput")
    r2d = nc.dram_tensor("r2t", [HD, HD], BF16, kind="ExternalInput")
    outd = nc.dram_tensor("out", [S, HID], BF16, kind="ExternalOutput")

    xT_t = xT.rearrange("(ko p) s -> p ko s", p=128)       # [128, 16, 2048]
    wqkv_t = wqkvT.rearrange("(ko p) o -> p ko o", p=128)  # [128, 16, 512]
    wo_t = woT.rearrange("(g p) h -> p g h", p=128)        # [128, 2, 2048]

    with tile.TileContext(nc) as tc, ExitStack() as ctx:
        # ---- permanent pools ----------------------------------------------
        consts = ctx.enter_context(tc.tile_pool(name="consts", bufs=1))
        persist = ctx.enter_context(tc.tile_pool(name="persist", bufs=1))
        # phase-2 SBUF pools opened before phase-1 pools so their memory
        # never overlaps (no WAR serialization at the phase boundary)
        p2s = ctx.enter_context(tc.tile_pool(name="p2s", bufs=2))
        expp = ctx.enter_context(tc.tile_pool(name="expp", bufs=4))
        outp = ctx.enter_context(tc.tile_pool(name="outp", bufs=3))

        # ---- persistent activations ---------------------------------------
        qTs = [persist.tile([128, S], BF16, tag=f"qT{h}", name=f"qT{h}")
               for h in range(QH_PER_CORE)]
        kT = persist.tile([128, S], BF16, tag="kT")
        v_sd = persist.tile([128, NST, HD], BF16, tag="v_sd")
        aoTs = [persist.tile([128, S], BF16, tag=f"aoT{h}", name=f"aoT{h}")
                for h in range(QH_PER_CORE)]
        xsb = persist.tile([128, NKC, S], BF16, tag="xsb")

        # ---- constants -----------------------------------------------------
        ident = consts.tile([128, 128], BF16)
        make_identity(nc, ident)

        # 0/1 causal mask for the diagonal 128x128 block of an expT tile:
        # keep (1) where sq >= sk i.e. col >= row, else 0
        mask01 = consts.tile([128, 128], BF16)
        nc.gpsimd.memset(mask01, 1.0)
        nc.gpsimd.affine_select(
            out=mask01,
            in_=mask01,
            compare_op=mybir.AluOpType.is_ge,
            fill=0.0,
            base=0,
            pattern=[[1, 128]],       # + 1*col
            channel_multiplier=-1,    # - row
        )

        ones_col = consts.tile([128, 1], BF16)
        nc.vector.memset(ones_col, 1.0)
        # f32 ones rows at partitions 0 and 32 (lhsT of the den broadcast)
        ones_f32 = consts.tile([64, 128], F32)
        nc.vector.memset(ones_f32, 1.0)

        r1_sb = consts.tile([128, 128], BF16)
        r2_sb = consts.tile([128, 128], BF16)
        wo_sb = consts.tile([128, QH_PER_CORE, HID], BF16)
        c1_sb = consts.tile([128, S], BF16)
        c2_sb = consts.tile([128, S], BF16)

        # ================= phase 1: projections + RoPE =====================
        with tc.tile_pool(name="p1c", bufs=1) as p1c, \
             tc.tile_pool(name="p1s", bufs=8) as p1s, \
             tc.tile_pool(name="p1t", bufs=2) as p1t, \
             tc.tile_pool(name="ps_proj", bufs=5, space="PSUM") as ps_proj, \
             tc.tile_pool(name="ps_rope", bufs=1, space="PSUM") as ps_rope, \
             tc.tile_pool(name="ps_vt", bufs=1, space="PSUM") as ps_vt:

            wqkv_sb = p1c.tile([128, NKC, 512], BF16)
            # chunk-0 x exclusively on the scalar HW queue (kc-paired to
            # amortize the ~600ns issue cost), weights on the sync HW queue
            # (4-kc groups); later x chunks follow on both queues.
            for kc in range(0, NKC, 2):
                nc.scalar.dma_start(xsb[:, kc:kc + 2, 0:SC],
                                    xT_t[:, kc:kc + 2, 0:SC])
            for kc in range(0, NKC, 4):
                nc.sync.dma_start(wqkv_sb[:, kc:kc + 4, :],
                                  wqkv_t[:, kc:kc + 4, :])
            for j in range(1, NSC):
                csl = slice(j * SC, (j + 1) * SC)
                for kc in range(0, NKC, 2):
                    eng = nc.scalar if (j + kc // 2) % 2 == 0 else nc.sync
                    eng.dma_start(xsb[:, kc:kc + 2, csl],
                                  xT_t[:, kc:kc + 2, csl])
            # constants on the gpsimd (SW) queue - off the critical path
            nc.gpsimd.dma_start(c1_sb, c1d[:, :])
            nc.gpsimd.dma_start(c2_sb, c2d[:, :])
            nc.gpsimd.dma_start(r1_sb, r1d[:, :])
            nc.gpsimd.dma_start(r2_sb, r2d[:, :])
            for g in range(QH_PER_CORE):
                nc.gpsimd.dma_start(wo_sb[:, g, :], wo_t[:, g, :])

            def rope_into(dst_chunk, raw, j):
                """dst = C1*(R1@raw) + C2*(R2@raw)."""
                ps_u = ps_rope.tile([128, SC], F32, tag="rope_u", name="ps_u")
                ps_w = ps_rope.tile([128, SC], F32, tag="rope_w", name="ps_w")
                nc.tensor.matmul(ps_u, r1_sb, raw, start=True, stop=True)
                nc.tensor.matmul(ps_w, r2_sb, raw, start=True, stop=True)
                csl = slice(j * SC, (j + 1) * SC)
                t1 = p1t.tile([128, SC], F32, tag="rope_t1", name="t1")
                t2 = p1t.tile([128, SC], F32, tag="rope_t2", name="t2")
                nc.vector.tensor_mul(out=t1, in0=ps_u, in1=c1_sb[:, csl])
                nc.vector.tensor_mul(out=t2, in0=ps_w, in1=c2_sb[:, csl])
                nc.vector.tensor_add(out=dst_chunk, in0=t1, in1=t2)

            def consume(j, raws):
                csl = slice(j * SC, (j + 1) * SC)
                for t in range(QH_PER_CORE):
                    rope_into(qTs[t][:, csl], raws[t], j)
                rope_into(kT[:, csl], raws[2], j)
                for b in range(SC // 128):
                    st = j * (SC // 128) + b
                    ps_t = ps_vt.tile([128, 128], BF16, tag="vtp", name="ps_t")
                    nc.tensor.transpose(
                        ps_t, raws[3][:, b * 128:(b + 1) * 128], ident
                    )
                    if b % 2 == 0:
                        nc.scalar.copy(out=v_sd[:, st, :], in_=ps_t)
                    else:
                        nc.vector.tensor_copy(out=v_sd[:, st, :], in_=ps_t)

            pend = None
            for j in range(NSC):
                csl = slice(j * SC, (j + 1) * SC)
                ps_ps = [ps_proj.tile([128, SC], F32, tag="mm", name=f"pp{t}")
                         for t in range(4)]
                for kc in range(NKC):
                    for t in range(4):  # q0, q1, k, v (columns of wqkv)
                        nc.tensor.matmul(
                            ps_ps[t],
                            wqkv_sb[:, kc, t * 128:(t + 1) * 128],
                            xsb[:, kc, csl],
                            start=(kc == 0),
                            stop=(kc == NKC - 1),
                        )
                raws = [p1s.tile([128, SC], BF16, tag="raw", name=f"raw{t}")
                        for t in range(4)]
                for t in range(4):
                    if t % 2 == 0:
                        nc.scalar.copy(out=raws[t], in_=ps_ps[t])
                    else:
                        nc.vector.tensor_copy(out=raws[t], in_=ps_ps[t])
                if pend is not None:
                    consume(*pend)
                pend = (j, raws)
            consume(*pend)

        # ============ phase 2: attention + output projection ===============
        with tc.tile_pool(name="ps_mm", bufs=3, space="PSUM") as ps_mm, \
             tc.tile_pool(name="ps_acc", bufs=2, space="PSUM") as ps_acc, \
             tc.tile_pool(name="ps_den", bufs=1, space="PSUM") as ps_den, \
             tc.tile_pool(name="ps_outc", bufs=2, space="PSUM") as ps_outc:

            chunk_state = {}

            def attention_chunk(j):
                csl = slice(j * SC, (j + 1) * SC)
                nk = 4 * j + 4  # causal: sk tiles 0..4j+3
                qcs = [qTs[h][:, csl] for h in range(QH_PER_CORE)]
                ps_os = [ps_acc.tile([128, SC], F32, tag="attn",
                                     name=f"ps_o{h}")
                         for h in range(QH_PER_CORE)]
                ps_dall = ps_den.tile([64, SC], F32, tag="den", name="ps_dall")
                ps_ds = [ps_dall[32 * h:32 * h + 1, :]
                         for h in range(QH_PER_CORE)]
                # software-pipelined: scores/exp for step k are emitted
                # before attnV/den of step k-1, so the PE never waits on the
                # ACT exp latency
                pend = None  # (k, vsl, e_tiles)

                def flush(pk, pvsl, pes, last):
                    # AV pair first, then the den pair adjacent (the two den
                    # matmuls hit col groups 0/1 and run concurrently);
                    # on the last step den goes first so the recip chain
                    # starts while the final AV pair still streams
                    av = [(h, 'av') for h in range(QH_PER_CORE)]
                    dn = [(h, 'dn') for h in range(QH_PER_CORE)]
                    order = dn + av if last else av + dn
                    for h, kind in order:
                        if kind == 'av':
                            nc.tensor.matmul(
                                ps_os[h][:, pvsl], v_sd[:, pk, :],
                                pes[h][:, pvsl],
                                start=(pk == 0), stop=last,
                            )
                        else:
                            nc.tensor.matmul(
                                ps_ds[h][:, pvsl], ones_col, pes[h][:, pvsl],
                                start=(pk == 0), stop=last,
                            )

                for k in range(nk):
                    m = k - 4 * j
                    # diagonal tiles (m>=0) only touch cols >= 128m
                    v0 = max(m, 0) * 128
                    vsl = slice(v0, SC)
                    kc_t = kT[:, k * 128:(k + 1) * 128]
                    es = []
                    for h in range(QH_PER_CORE):
                        ps_s = ps_mm.tile([128, SC], F32, tag="mm",
                                          name="ps_s")
                        nc.tensor.matmul(
                            ps_s[:, vsl], kc_t, qcs[h][:, vsl],
                            start=True, stop=True,
                        )
                        e = expp.tile([128, SC], BF16, tag="exp", name="e")
                        nc.scalar.activation(
                            out=e[:, vsl], in_=ps_s[:, vsl],
                            func=mybir.ActivationFunctionType.Exp,
                            scale=SCALE,
                        )
                        if m >= 0:
                            dsl = slice(m * 128, (m + 1) * 128)
                            nc.vector.tensor_mul(
                                out=e[:, dsl], in0=e[:, dsl], in1=mask01,
                            )
                        es.append(e)
                    if pend is not None:
                        flush(*pend, last=False)
                    pend = (k, vsl, es)
                flush(*pend, last=True)
                chunk_state[j] = (ps_os, ps_dall)

            def norm_chunk(j):
                """den -> recip (DVE) -> PE broadcast -> normalize into aoT."""
                csl = slice(j * SC, (j + 1) * SC)
                ps_os, ps_dall = chunk_state.pop(j)
                recip = p2s.tile([64, SC], F32, tag="recip", name="recip")
                nc.vector.reciprocal_approx_fast(out=recip, in_=ps_dall)
                for h in range(QH_PER_CORE):
                    db_ps = ps_mm.tile([128, SC], F32, tag="mm",
                                       name=f"db{h}")
                    nc.tensor.matmul(
                        db_ps,
                        ones_f32[32 * h:32 * h + 1, :],
                        recip[32 * h:32 * h + 1, :],
                        start=True, stop=True,
                    )
                    db_s = p2s.tile([128, SC], F32, tag="db_s",
                                    name=f"dbs{h}")
                    nc.vector.tensor_copy(out=db_s, in_=db_ps)
                    nc.vector.tensor_mul(out=aoTs[h][:, csl],
                                         in0=ps_os[h], in1=db_s)

            def outproj_st(j, b):
                """one 128-row block of the output projection of chunk j."""
                st = j * (SC // 128) + b
                ssl = slice(st * 128, (st + 1) * 128)
                ob = outp.tile([128, HID], BF16, tag="outb", name="ob")
                for hc in range(NSC):
                    hsl = slice(hc * SC, (hc + 1) * SC)
                    ps_c = ps_outc.tile([128, SC], F32, tag="oc",
                                        name="ps_c")
                    for h in range(QH_PER_CORE):
                        nc.tensor.matmul(
                            ps_c,
                            aoTs[h][:, ssl],
                            wo_sb[:, h, hsl],
                            start=(h == 0),
                            stop=(h == QH_PER_CORE - 1),
                        )
                    if hc % 2 == 0:
                        nc.scalar.copy(out=ob[:, hsl], in_=ps_c)
                    else:
                        nc.vector.tensor_copy(out=ob[:, hsl], in_=ps_c)
                eng = nc.sync if st % 2 == 0 else nc.scalar
                eng.dma_start(outd[ssl, :], ob)

            attention_chunk(0)
            norm_chunk(0)
            for j in range(1, NSC):
                attention_chunk(j)
                outproj_st(j - 1, 0)
                norm_chunk(j)
                for b in range(1, SC // 128):
                    outproj_st(j - 1, b)
            for b in range(SC // 128):
                outproj_st(NSC - 1, b)

    nc.finalize()
    return nc


def shard_inputs(x, cos, sin, wq, wk, wv, wo):
    x = np.asarray(x, np.float32).reshape(S, HID)
    cos = np.asarray(cos, np.float32)
    sin = np.asarray(sin, np.float32)
    wq = np.asarray(wq, np.float32)
    wk = np.asarray(wk, np.float32)
    wv = np.asarray(wv, np.float32)
    wo = np.asarray(wo, np.float32)

    xT = np.ascontiguousarray(x.T).astype(NPBF)

    cos_h, sin_h = cos[:, :HH].T, sin[:, :HH].T       # [64, S]
    c1 = np.ascontiguousarray(
        np.concatenate([cos_h, -sin_h], axis=0)).astype(NPBF)
    c2 = np.ascontiguousarray(
        np.concatenate([sin_h, cos_h], axis=0)).astype(NPBF)

    r1 = np.zeros((HD, HD), np.float32)
    for i in range(HH // 2):
        r1[2 * i, 2 * i + 1] = -1.0
        r1[2 * i + 1, 2 * i] = 1.0
    r1[HH:, :] = r1[:HH, :]
    r2 = np.zeros((HD, HD), np.float32)
    for d in range(HH):
        r2[d, d + HH] = 1.0
        r2[d + HH, d + HH] = 1.0
    r1t = np.ascontiguousarray(r1.T).astype(NPBF)  # lhsT for out = R1 @ rhs
    r2t = np.ascontiguousarray(r2.T).astype(NPBF)

    in_maps = []
    for c in range(NCORES):
        h0 = QH_PER_CORE * c
        kvh = h0 * NKV // NH
        wq_c = wq[h0 * HD:(h0 + QH_PER_CORE) * HD, :]    # [256, HID]
        wk_c = wk[kvh * HD:(kvh + 1) * HD, :]            # [128, HID]
        wv_c = wv[kvh * HD:(kvh + 1) * HD, :]
        wqkvT_c = np.ascontiguousarray(
            np.concatenate([wq_c, wk_c, wv_c], axis=0).T).astype(NPBF)
        woT_c = np.ascontiguousarray(
            wo[:, h0 * HD:(h0 + QH_PER_CORE) * HD].T).astype(NPBF)
        in_maps.append({
            "xt": xT,
            "wqkvt": wqkvT_c,
            "wot": woT_c,
            "c1": c1,
            "c2": c2,
            "r1t": r1t,
            "r2t": r2t,
        })
    return in_maps


_CACHED_NC = None


def kernel(x, cos, sin, wq, wk, wv, wo, _trace=False, _tmpdir=None):
    global _CACHED_NC
    in_maps = shard_inputs(x, cos, sin, wq, wk, wv, wo)
    if _CACHED_NC is None:
        _CACHED_NC = build_graph()
    nc = _CACHED_NC
    res = bass_utils.run_bass_kernel_spmd(
        nc, in_maps, core_ids=list(range(NCORES)),
        trace=_trace, tmpdir=_tmpdir,
    )
    total = np.zeros((S, HID), np.float32)
    for r in res.results:
        total += np.asarray(r["out"], dtype=np.float32)
    out = total.reshape(1, S, HID)
    if _trace:
        return out, res
    return out


# revision 15
# speedup vs baseline: 1.1525x; 1.0029x over previous
"""Trainium2 Bass kernel for RoPE + GQA causal attention (B=1, S=2048, HID=2048,
NH=16, NKV=4, HD=128), tensor-parallel over heads across 8 NeuronCores.

Sharding: core c computes q heads {2c, 2c+1} and kv head c//2, plus the
corresponding slice of the output projection (wo input-dim shard). Each core
emits a partial [S, HID] output in bf16; the host sums the 8 partials.

Per-core dataflow (transposed layout, d on partitions; bf16 matmuls, f32 PSUM):
  phase 1 (kc-major projections, software-pipelined RoPE/V-transpose):
    xT[h,s] -> QT/KT/VT = W^T-slices @ xT     (4 PSUM accumulators, kc outer)
    RoPE(qT) = C1 * (R1 @ qT) + C2 * (R2 @ qT)
    V transposed to [s,d] tiles via PE transpose
  phase 2 (attention + lagged output projection):
    scoresT[sk,sq] = KT^T-tile @ QT-chunk     -> exp on ACT (only ACT func)
    causal mask applied as 0/1 multiply on the exp tile (DVE)
    attnT[d,sq] += V-tile @ expT              (PSUM accum over sk tiles)
    den[1,sq]  += ones^T @ expT               (pairs col-packed, partitions
                                               0/32 -> concurrent col groups)
    recip = reciprocal_approx_fast(den)       (single DVE op, no ACT tables)
    db = ones_row^T @ recip                   (PE partition-broadcast)
    aoT = attnT * db                          (DVE, writes bf16)
    partial[s,h] = aoT-slices^T @ woT         (batched [128,HID] bf16 out DMA)
"""

import os
import sys
from contextlib import ExitStack

for _p in ("/opt/trn_rl_repo", "/root/.axon_site/_ro/trn_rl_repo"):
    if os.path.isdir(_p) and _p not in sys.path:
        sys.path.append(_p)

import ml_dtypes
import numpy as np

import concourse.bass as bass
import concourse.mybir as mybir
import concourse.tile as tile
from concourse import bacc, bass_utils
from concourse.masks import make_identity

S, HID, NH, NKV, HD = 2048, 2048, 16, 4, 128
HH = HD // 2  # 64
NCORES = 8
QH_PER_CORE = NH // NCORES  # 2
SCALE = float(1.0 / np.sqrt(HD))

F32 = mybir.dt.float32
BF16 = mybir.dt.bfloat16
NPBF = ml_dtypes.bfloat16

SC = 512          # s-chunk width (free dim of most matmuls)
NSC = S // SC     # 4
NKC = HID // 128  # 16 contraction chunks
NST = S // 128    # 16 s-tiles


def build_graph():
    nc = bacc.Bacc(trn_type="TRN2", enable_partition_id=False)

    # x and wqkv are pre-transposed on the host into SBUF-partition-major
    # layouts so each partition's DMA read is a fat contiguous block
    # (16KB/partition) instead of 1KB lines.
    xt_d = nc.dram_tensor("xt", [NSC, 128, NKC, SC], BF16,
                          kind="ExternalInput")
    wqkv_d = nc.dram_tensor("wqkvt", [128, NKC, 512], BF16,
                            kind="ExternalInput")
    woT = nc.dram_tensor("wot", [QH_PER_CORE * HD, HID], BF16, kind="ExternalInput")
    c1d = nc.dram_tensor("c1", [HD, S], BF16, kind="ExternalInput")
    c2d = nc.dram_tensor("c2", [HD, S], BF16, kind="ExternalInput")
    r1d = nc.dram_tensor("r1t", [HD, HD], BF16, kind="ExternalInput")
    r2d = nc.dram_tensor("r2t", [HD, HD], BF16, kind="ExternalInput")
    outd = nc.dram_tensor("out", [S, HID], BF16, kind="ExternalOutput")

    wo_t = woT.rearrange("(g p) h -> p g h", p=128)        # [128, 2, 2048]

    with tile.TileContext(nc) as tc, ExitStack() as ctx:
        # ---- permanent pools ----------------------------------------------
        consts = ctx.enter_context(tc.tile_pool(name="consts", bufs=1))
        persist = ctx.enter_context(tc.tile_pool(name="persist", bufs=1))
        # phase-2 SBUF pools opened before phase-1 pools so their memory
        # never overlaps (no WAR serialization at the phase boundary)
        p2s = ctx.enter_context(tc.tile_pool(name="p2s", bufs=2))
        expp = ctx.enter_context(tc.tile_pool(name="expp", bufs=4))
        outp = ctx.enter_context(tc.tile_pool(name="outp", bufs=3))

        # ---- persistent activations ---------------------------------------
        qTs = [persist.tile([128, S], BF16, tag=f"qT{h}", name=f"qT{h}")
               for h in range(QH_PER_CORE)]
        kT = persist.tile([128, S], BF16, tag="kT")
        v_sd = persist.tile([128, NST, HD], BF16, tag="v_sd")
        aoTs = [persist.tile([128, S], BF16, tag=f"aoT{h}", name=f"aoT{h}")
                for h in range(QH_PER_CORE)]
        xsb = persist.tile([128, NKC, S], BF16, tag="xsb")

        # ---- constants -----------------------------------------------------
        ident = consts.tile([128, 128], BF16)
        make_identity(nc, ident)

        # 0/1 causal mask for the diagonal 128x128 block of an expT tile:
        # keep (1) where sq >= sk i.e. col >= row, else 0
        mask01 = consts.tile([128, 128], BF16)
        nc.gpsimd.memset(mask01, 1.0)
        nc.gpsimd.affine_select(
            out=mask01,
            in_=mask01,
            compare_op=mybir.AluOpType.is_ge,
            fill=0.0,
            base=0,
            pattern=[[1, 128]],       # + 1*col
            channel_multiplier=-1,    # - row
        )

        ones_col = consts.tile([128, 1], BF16)
        nc.vector.memset(ones_col, 1.0)
        # f32 ones rows at partitions 0 and 32 (lhsT of the den broadcast)
        ones_f32 = consts.tile([64, 128], F32)
        nc.vector.memset(ones_f32, 1.0)

        r1_sb = consts.tile([128, 128], BF16)
        r2_sb = consts.tile([128, 128], BF16)
        wo_sb = consts.tile([128, QH_PER_CORE, HID], BF16)
        c1_sb = consts.tile([128, S], BF16)
        c2_sb = consts.tile([128, S], BF16)

        # ================= phase 1: projections + RoPE =====================
        with tc.tile_pool(name="p1c", bufs=1) as p1c, \
             tc.tile_pool(name="p1s", bufs=8) as p1s, \
             tc.tile_pool(name="p1t", bufs=2) as p1t, \
             tc.tile_pool(name="ps_proj", bufs=5, space="PSUM") as ps_proj, \
             tc.tile_pool(name="ps_rope", bufs=1, space="PSUM") as ps_rope, \
             tc.tile_pool(name="ps_vt", bufs=1, space="PSUM") as ps_vt:

            wqkv_sb = p1c.tile([128, NKC, 512], BF16)
            # chunk-0 x exclusively on the scalar HW queue in escalating
            # kc groups, weights on the sync HW queue; later x chunks
            # follow split across both queues. All reads are fat
            # (4-16KB/partition contiguous) thanks to the host layout.
            for k0, k1 in ((0, 2), (2, 4), (4, 8), (8, 16)):
                nc.scalar.dma_start(xsb[:, k0:k1, 0:SC],
                                    xt_d[0, :, k0:k1, :])
            nc.sync.dma_start(wqkv_sb[:, 0:8, :], wqkv_d[:, 0:8, :])
            nc.sync.dma_start(wqkv_sb[:, 8:16, :], wqkv_d[:, 8:16, :])
            for j in range(1, NSC):
                csl = slice(j * SC, (j + 1) * SC)
                for k0, k1 in ((0, 8), (8, 16)):
                    eng = nc.scalar if (j + k0 // 8) % 2 == 0 else nc.sync
                    eng.dma_start(xsb[:, k0:k1, csl],
                                  xt_d[j, :, k0:k1, :])
            # constants on the gpsimd (SW) queue - off the critical path
            nc.gpsimd.dma_start(c1_sb, c1d[:, :])
            nc.gpsimd.dma_start(c2_sb, c2d[:, :])
            nc.gpsimd.dma_start(r1_sb, r1d[:, :])
            nc.gpsimd.dma_start(r2_sb, r2d[:, :])
            for g in range(QH_PER_CORE):
                nc.gpsimd.dma_start(wo_sb[:, g, :], wo_t[:, g, :])

            def rope_into(dst_chunk, raw, j):
                """dst = C1*(R1@raw) + C2*(R2@raw)."""
                ps_u = ps_rope.tile([128, SC], F32, tag="rope_u", name="ps_u")
                ps_w = ps_rope.tile([128, SC], F32, tag="rope_w", name="ps_w")
                nc.tensor.matmul(ps_u, r1_sb, raw, start=True, stop=True)
                nc.tensor.matmul(ps_w, r2_sb, raw, start=True, stop=True)
                csl = slice(j * SC, (j + 1) * SC)
                t1 = p1t.tile([128, SC], F32, tag="rope_t1", name="t1")
                t2 = p1t.tile([128, SC], F32, tag="rope_t2", name="t2")
                nc.vector.tensor_mul(out=t1, in0=ps_u, in1=c1_sb[:, csl])
                nc.vector.tensor_mul(out=t2, in0=ps_w, in1=c2_sb[:, csl])
                nc.vector.tensor_add(out=dst_chunk, in0=t1, in1=t2)

            def consume(j, raws):
                csl = slice(j * SC, (j + 1) * SC)
                for t in range(QH_PER_CORE):
                    rope_into(qTs[t][:, csl], raws[t], j)
                rope_into(kT[:, csl], raws[2], j)
                for b in range(SC // 128):
                    st = j * (SC // 128) + b
                    ps_t = ps_vt.tile([128, 128], BF16, tag="vtp", name="ps_t")
                    nc.tensor.transpose(
                        ps_t, raws[3][:, b * 128:(b + 1) * 128], ident
                    )
                    if b % 2 == 0:
                        nc.scalar.copy(out=v_sd[:, st, :], in_=ps_t)
                    else:
                        nc.vector.tensor_copy(out=v_sd[:, st, :], in_=ps_t)

            pend = None
            for j in range(NSC):
                csl = slice(j * SC, (j + 1) * SC)
                ps_ps = [ps_proj.tile([128, SC], F32, tag="mm", name=f"pp{t}")
                         for t in range(4)]
                for kc in range(NKC):
                    for t in range(4):  # q0, q1, k, v (columns of wqkv)
                        nc.tensor.matmul(
                            ps_ps[t],
                            wqkv_sb[:, kc, t * 128:(t + 1) * 128],
                            xsb[:, kc, csl],
                            start=(kc == 0),
                            stop=(kc == NKC - 1),
                        )
                raws = [p1s.tile([128, SC], BF16, tag="raw", name=f"raw{t}")
                        for t in range(4)]
                for t in range(4):
                    if t % 2 == 0:
                        nc.scalar.copy(out=raws[t], in_=ps_ps[t])
                    else:
                        nc.vector.tensor_copy(out=raws[t], in_=ps_ps[t])
                if pend is not None:
                    consume(*pend)
                pend = (j, raws)
            consume(*pend)

        # ============ phase 2: attention + output projection ===============
        with tc.tile_pool(name="ps_mm", bufs=3, space="PSUM") as ps_mm, \
             tc.tile_pool(name="ps_acc", bufs=2, space="PSUM") as ps_acc, \
             tc.tile_pool(name="ps_den", bufs=1, space="PSUM") as ps_den, \
             tc.tile_pool(name="ps_outc", bufs=2, space="PSUM") as ps_outc:

            chunk_state = {}

            def attention_chunk(j, fillers=()):
                fillers = list(fillers)
                csl = slice(j * SC, (j + 1) * SC)
                nk = 4 * j + 4  # causal: sk tiles 0..4j+3
                fill_at = set()
                if fillers:
                    step = max(1, (nk - 3) // len(fillers))
                    fill_at = {2 + i * step for i in range(len(fillers))}
                qcs = [qTs[h][:, csl] for h in range(QH_PER_CORE)]
                ps_os = [ps_acc.tile([128, SC], F32, tag="attn",
                                     name=f"ps_o{h}")
                         for h in range(QH_PER_CORE)]
                ps_dall = ps_den.tile([64, SC], F32, tag="den", name="ps_dall")
                ps_ds = [ps_dall[32 * h:32 * h + 1, :]
                         for h in range(QH_PER_CORE)]
                # software-pipelined: scores/exp for step k are emitted
                # before attnV/den of step k-1, so the PE never waits on the
                # ACT exp latency
                pend = None  # (k, vsl, e_tiles)

                def flush(pk, pvsl, pes, last):
                    # AV pair first, then the den pair adjacent (the two den
                    # matmuls hit col groups 0/1 and run concurrently);
                    # on the last step den goes first so the recip chain
                    # starts while the final AV pair still streams
                    av = [(h, 'av') for h in range(QH_PER_CORE)]
                    dn = [(h, 'dn') for h in range(QH_PER_CORE)]
                    order = dn + av if last else av + dn
                    for h, kind in order:
                        if kind == 'av':
                            nc.tensor.matmul(
                                ps_os[h][:, pvsl], v_sd[:, pk, :],
                                pes[h][:, pvsl],
                                start=(pk == 0), stop=last,
                            )
                        else:
                            nc.tensor.matmul(
                                ps_ds[h][:, pvsl], ones_col, pes[h][:, pvsl],
                                start=(pk == 0), stop=last,
                            )

                for k in range(nk):
                    m = k - 4 * j
                    # diagonal tiles (m>=0) only touch cols >= 128m
                    v0 = max(m, 0) * 128
                    vsl = slice(v0, SC)
                    kc_t = kT[:, k * 128:(k + 1) * 128]
                    es = []
                    for h in range(QH_PER_CORE):
                        ps_s = ps_mm.tile([128, SC], F32, tag="mm",
                                          name="ps_s")
                        nc.tensor.matmul(
                            ps_s[:, vsl], kc_t, qcs[h][:, vsl],
                            start=True, stop=True,
                        )
                        e = expp.tile([128, SC], BF16, tag="exp", name="e")
                        nc.scalar.activation(
                            out=e[:, vsl], in_=ps_s[:, vsl],
                            func=mybir.ActivationFunctionType.Exp,
                            scale=SCALE,
                        )
                        if m >= 0:
                            dsl = slice(m * 128, (m + 1) * 128)
                            nc.vector.tensor_mul(
                                out=e[:, dsl], in0=e[:, dsl], in1=mask01,
                            )
                        es.append(e)
                    if pend is not None:
                        flush(*pend, last=False)
                    pend = (k, vsl, es)
                    if k in fill_at and fillers:
                        fillers.pop(0)()
                flush(*pend, last=True)
                for f in fillers:
                    f()
                chunk_state[j] = (ps_os, ps_dall)

            def norm_chunk(j):
                """den -> recip (DVE) -> PE broadcast -> normalize into aoT."""
                csl = slice(j * SC, (j + 1) * SC)
                ps_os, ps_dall = chunk_state.pop(j)
                recip = p2s.tile([64, SC], F32, tag="recip", name="recip")
                nc.vector.reciprocal_approx_fast(out=recip, in_=ps_dall)
                for h in range(QH_PER_CORE):
                    db_ps = ps_mm.tile([128, SC], F32, tag="mm",
                                       name=f"db{h}")
                    nc.tensor.matmul(
                        db_ps,
                        ones_f32[32 * h:32 * h + 1, :],
                        recip[32 * h:32 * h + 1, :],
                        start=True, stop=True,
                    )
                    db_s = p2s.tile([128, SC], F32, tag="db_s",
                                    name=f"dbs{h}")
                    nc.vector.tensor_copy(out=db_s, in_=db_ps)
                    nc.vector.tensor_mul(out=aoTs[h][:, csl],
                                         in0=ps_os[h], in1=db_s)

            def outproj_st(j, b):
                """one 128-row block of the output projection of chunk j."""
                st = j * (SC // 128) + b
                ssl = slice(st * 128, (st + 1) * 128)
                ob = outp.tile([128, HID], BF16, tag="outb", name="ob")
                for hc in range(NSC):
                    hsl = slice(hc * SC, (hc + 1) * SC)
                    ps_c = ps_outc.tile([128, SC], F32, tag="oc",
                                        name="ps_c")
                    for h in range(QH_PER_CORE):
                        nc.tensor.matmul(
                            ps_c,
                            aoTs[h][:, ssl],
                            wo_sb[:, h, hsl],
                            start=(h == 0),
                            stop=(h == QH_PER_CORE - 1),
                        )
                    if hc % 2 == 0:
                        nc.scalar.copy(out=ob[:, hsl], in_=ps_c)
                    else:
                        nc.vector.tensor_copy(out=ob[:, hsl], in_=ps_c)
                eng = nc.sync if st % 2 == 0 else nc.scalar
                eng.dma_start(outd[ssl, :], ob)

            attention_chunk(0)
            norm_chunk(0)
            for j in range(1, NSC):
                fillers = [
                    (lambda jj, bb: lambda: outproj_st(jj, bb))(j - 1, b)
                    for b in range(SC // 128)
                ]
                attention_chunk(j, fillers)
                norm_chunk(j)
            for b in range(SC // 128):
                outproj_st(NSC - 1, b)

    nc.finalize()
    return nc


def shard_inputs(x, cos, sin, wq, wk, wv, wo):
    x = np.asarray(x, np.float32).reshape(S, HID)
    cos = np.asarray(cos, np.float32)
    sin = np.asarray(sin, np.float32)
    wq = np.asarray(wq, np.float32)
    wk = np.asarray(wk, np.float32)
    wv = np.asarray(wv, np.float32)
    wo = np.asarray(wo, np.float32)

    # [NSC, 128, NKC, SC]: per (chunk, partition) a contiguous 16KB block
    xT = x.T.astype(NPBF)  # [HID, S]
    xt_fat = np.ascontiguousarray(
        xT.reshape(NKC, 128, NSC, SC).transpose(2, 1, 0, 3))

    cos_h, sin_h = cos[:, :HH].T, sin[:, :HH].T       # [64, S]
    c1 = np.ascontiguousarray(
        np.concatenate([cos_h, -sin_h], axis=0)).astype(NPBF)
    c2 = np.ascontiguousarray(
        np.concatenate([sin_h, cos_h], axis=0)).astype(NPBF)

    r1 = np.zeros((HD, HD), np.float32)
    for i in range(HH // 2):
        r1[2 * i, 2 * i + 1] = -1.0
        r1[2 * i + 1, 2 * i] = 1.0
    r1[HH:, :] = r1[:HH, :]
    r2 = np.zeros((HD, HD), np.float32)
    for d in range(HH):
        r2[d, d + HH] = 1.0
        r2[d + HH, d + HH] = 1.0
    r1t = np.ascontiguousarray(r1.T).astype(NPBF)  # lhsT for out = R1 @ rhs
    r2t = np.ascontiguousarray(r2.T).astype(NPBF)

    in_maps = []
    for c in range(NCORES):
        h0 = QH_PER_CORE * c
        kvh = h0 * NKV // NH
        wq_c = wq[h0 * HD:(h0 + QH_PER_CORE) * HD, :]    # [256, HID]
        wk_c = wk[kvh * HD:(kvh + 1) * HD, :]            # [128, HID]
        wv_c = wv[kvh * HD:(kvh + 1) * HD, :]
        wqkvT_c = np.concatenate([wq_c, wk_c, wv_c], axis=0).T.astype(NPBF)
        # [128, NKC, 512]: per partition a contiguous 16KB block
        wqkv_fat = np.ascontiguousarray(
            wqkvT_c.reshape(NKC, 128, 512).transpose(1, 0, 2))
        woT_c = np.ascontiguousarray(
            wo[:, h0 * HD:(h0 + QH_PER_CORE) * HD].T).astype(NPBF)
        in_maps.append({
            "xt": xt_fat,
            "wqkvt": wqkv_fat,
            "wot": woT_c,
            "c1": c1,
            "c2": c2,
            "r1t": r1t,
            "r2t": r2t,
        })
    return in_maps


_CACHED_NC = None


def kernel(x, cos, sin, wq, wk, wv, wo, _trace=False, _tmpdir=None):
    global _CACHED_NC
    in_maps = shard_inputs(x, cos, sin, wq, wk, wv, wo)
    if _CACHED_NC is None:
        _CACHED_NC = build_graph()
    nc = _CACHED_NC
    res = bass_utils.run_bass_kernel_spmd(
        nc, in_maps, core_ids=list(range(NCORES)),
        trace=_trace, tmpdir=_tmpdir,
    )
    total = np.zeros((S, HID), np.float32)
    for r in res.results:
        total += np.asarray(r["out"], dtype=np.float32)
    out = total.reshape(1, S, HID)
    if _trace:
        return out, res
    return out


# revision 21
# speedup vs baseline: 1.1661x; 1.0118x over previous
"""Trainium2 Bass kernel for RoPE + GQA causal attention (B=1, S=2048, HID=2048,
NH=16, NKV=4, HD=128), tensor-parallel over heads across 8 NeuronCores.

Sharding: core c computes q heads {2c, 2c+1} and kv head c//2, plus the
corresponding slice of the output projection (wo input-dim shard). Each core
emits a partial [S, HID] output in bf16; the host sums the 8 partials.

Per-core dataflow (transposed layout, d on partitions; bf16 matmuls, f32 PSUM):
  phase 1 (kc-major projections, software-pipelined RoPE/V-transpose):
    xT[h,s] -> QT/KT/VT = W^T-slices @ xT     (4 PSUM accumulators, kc outer)
    RoPE(qT) = C1 * (R1 @ qT) + C2 * (R2 @ qT)
    V transposed to [s,d] tiles via PE transpose
  phase 2 (attention + lagged output projection):
    scoresT[sk,sq] = KT^T-tile @ QT-chunk     -> exp on ACT (only ACT func)
    causal mask applied as 0/1 multiply on the exp tile (DVE)
    attnT[d,sq] += V-tile @ expT              (PSUM accum over sk tiles)
    den[1,sq]  += ones^T @ expT               (pairs col-packed, partitions
                                               0/32 -> concurrent col groups)
    recip = reciprocal_approx_fast(den)       (single DVE op, no ACT tables)
    db = ones_row^T @ recip                   (PE partition-broadcast)
    aoT = attnT * db                          (DVE, writes bf16)
    partial[s,h] = aoT-slices^T @ woT         (batched [128,HID] bf16 out DMA)
"""

import os
import sys
from contextlib import ExitStack

for _p in ("/opt/trn_rl_repo", "/root/.axon_site/_ro/trn_rl_repo"):
    if os.path.isdir(_p) and _p not in sys.path:
        sys.path.append(_p)

import ml_dtypes
import numpy as np

import concourse.bass as bass
import concourse.mybir as mybir
import concourse.tile as tile
from concourse import bacc, bass_utils
from concourse.masks import make_identity

S, HID, NH, NKV, HD = 2048, 2048, 16, 4, 128
HH = HD // 2  # 64
NCORES = 8
QH_PER_CORE = NH // NCORES  # 2
SCALE = float(1.0 / np.sqrt(HD))

F32 = mybir.dt.float32
BF16 = mybir.dt.bfloat16
NPBF = ml_dtypes.bfloat16

SC = 512          # s-chunk width (free dim of most matmuls)
NSC = S // SC     # 4
NKC = HID // 128  # 16 contraction chunks
NST = S // 128    # 16 s-tiles


def build_graph():
    nc = bacc.Bacc(trn_type="TRN2", enable_partition_id=False)

    # x and wqkv are pre-transposed on the host into SBUF-partition-major
    # layouts so each partition's DMA read is a fat contiguous block
    # (16KB/partition) instead of 1KB lines.
    xt_d = nc.dram_tensor("xt", [NSC, 128, NKC, SC], BF16,
                          kind="ExternalInput")
    wqkv_d = nc.dram_tensor("wqkvt", [128, NKC, 512], BF16,
                            kind="ExternalInput")
    woT = nc.dram_tensor("wot", [QH_PER_CORE * HD, HID], BF16, kind="ExternalInput")
    c1d = nc.dram_tensor("c1", [HD, S], BF16, kind="ExternalInput")
    c2d = nc.dram_tensor("c2", [HD, S], BF16, kind="ExternalInput")
    r1d = nc.dram_tensor("r1t", [HD, HD], BF16, kind="ExternalInput")
    r2d = nc.dram_tensor("r2t", [HD, HD], BF16, kind="ExternalInput")
    outd = nc.dram_tensor("out", [S, HID], BF16, kind="ExternalOutput")

    wo_t = woT.rearrange("(g p) h -> p g h", p=128)        # [128, 2, 2048]

    with tile.TileContext(nc) as tc, ExitStack() as ctx:
        # ---- permanent pools ----------------------------------------------
        consts = ctx.enter_context(tc.tile_pool(name="consts", bufs=1))
        persist = ctx.enter_context(tc.tile_pool(name="persist", bufs=1))
        # phase-2 SBUF pools opened before phase-1 pools so their memory
        # never overlaps (no WAR serialization at the phase boundary)
        p2s = ctx.enter_context(tc.tile_pool(name="p2s", bufs=2))
        expp = ctx.enter_context(tc.tile_pool(name="expp", bufs=4))
        outp = ctx.enter_context(tc.tile_pool(name="outp", bufs=3))

        # ---- persistent activations ---------------------------------------
        qTs = [persist.tile([128, S], BF16, tag=f"qT{h}", name=f"qT{h}")
               for h in range(QH_PER_CORE)]
        kT = persist.tile([128, S], BF16, tag="kT")
        v_sd = persist.tile([128, NST, HD], BF16, tag="v_sd")
        aoTs = [persist.tile([128, S], BF16, tag=f"aoT{h}", name=f"aoT{h}")
                for h in range(QH_PER_CORE)]
        xsb = persist.tile([128, NKC, S], BF16, tag="xsb")

        # ---- constants -----------------------------------------------------
        ident = consts.tile([128, 128], BF16)
        make_identity(nc, ident)

        # 0/1 causal mask for the diagonal 128x128 block of an expT tile:
        # keep (1) where sq >= sk i.e. col >= row, else 0
        mask01 = consts.tile([128, 128], BF16)
        nc.gpsimd.memset(mask01, 1.0)
        nc.gpsimd.affine_select(
            out=mask01,
            in_=mask01,
            compare_op=mybir.AluOpType.is_ge,
            fill=0.0,
            base=0,
            pattern=[[1, 128]],       # + 1*col
            channel_multiplier=-1,    # - row
        )

        ones_col = consts.tile([128, 1], BF16)
        nc.vector.memset(ones_col, 1.0)
        # f32 ones rows at partitions 0 and 32 (lhsT of the den broadcast)
        ones_f32 = consts.tile([64, 128], F32)
        nc.vector.memset(ones_f32, 1.0)

        r1_sb = consts.tile([128, 128], BF16)
        r2_sb = consts.tile([128, 128], BF16)
        wo_sb = consts.tile([128, QH_PER_CORE, HID], BF16)
        c1_sb = consts.tile([128, S], BF16)
        c2_sb = consts.tile([128, S], BF16)

        # ================= phase 1: projections + RoPE =====================
        with tc.tile_pool(name="p1c", bufs=1) as p1c, \
             tc.tile_pool(name="p1s", bufs=8) as p1s, \
             tc.tile_pool(name="p1t", bufs=2) as p1t, \
             tc.tile_pool(name="ps_proj", bufs=5, space="PSUM") as ps_proj, \
             tc.tile_pool(name="ps_ru", bufs=2, space="PSUM") as ps_ru, \
             tc.tile_pool(name="ps_rw", bufs=1, space="PSUM") as ps_rw:

            wqkv_sb = p1c.tile([128, NKC, 512], BF16)
            # chunk-0 x exclusively on the scalar HW queue in escalating
            # kc groups, weights on the sync HW queue; later x chunks
            # follow split across both queues. All reads are fat
            # (4-16KB/partition contiguous) thanks to the host layout.
            # chunk-0 inputs stream in kc-pairs round-robin across the three
            # DMA-capable queues (sync, scalar, gpsimd) in consumption order
            # so the kc-major accumulation never waits on a transfer.
            qs = (nc.sync, nc.scalar, nc.gpsimd)
            qi = 0
            for k0 in range(0, NKC, 2):
                qs[qi % 3].dma_start(wqkv_sb[:, k0:k0 + 2, :],
                                     wqkv_d[:, k0:k0 + 2, :])
                qs[(qi + 1) % 3].dma_start(xsb[:, k0:k0 + 2, 0:SC],
                                           xt_d[0, :, k0:k0 + 2, :])
                qi += 2
            for j in range(1, NSC):
                csl = slice(j * SC, (j + 1) * SC)
                for k0, k1 in ((0, 8), (8, 16)):
                    eng = nc.scalar if (j + k0 // 8) % 2 == 0 else nc.sync
                    eng.dma_start(xsb[:, k0:k1, csl],
                                  xt_d[j, :, k0:k1, :])
            # constants on the gpsimd queue tail - off the critical path
            nc.gpsimd.dma_start(r1_sb, r1d[:, :])
            nc.gpsimd.dma_start(r2_sb, r2d[:, :])
            nc.gpsimd.dma_start(c1_sb, c1d[:, :])
            nc.gpsimd.dma_start(c2_sb, c2d[:, :])
            for g in range(QH_PER_CORE):
                nc.gpsimd.dma_start(wo_sb[:, g, :], wo_t[:, g, :])

            def rope_into(dst_chunk, raw, j):
                """dst = C1*(R1@raw) + C2*(R2@raw)."""
                ps_u = ps_ru.tile([128, SC], F32, tag="rope_u", name="ps_u")
                ps_w = ps_rw.tile([128, SC], F32, tag="rope_w", name="ps_w")
                nc.tensor.matmul(ps_u, r1_sb, raw, start=True, stop=True)
                nc.tensor.matmul(ps_w, r2_sb, raw, start=True, stop=True)
                csl = slice(j * SC, (j + 1) * SC)
                t1 = p1t.tile([128, SC], F32, tag="rope_t1", name="t1")
                t2 = p1t.tile([128, SC], F32, tag="rope_t2", name="t2")
                nc.vector.tensor_mul(out=t1, in0=ps_u, in1=c1_sb[:, csl])
                nc.vector.tensor_mul(out=t2, in0=ps_w, in1=c2_sb[:, csl])
                nc.vector.tensor_add(out=dst_chunk, in0=t1, in1=t2)

            def consume(j, raws):
                csl = slice(j * SC, (j + 1) * SC)
                for t in range(QH_PER_CORE):
                    rope_into(qTs[t][:, csl], raws[t], j)
                rope_into(kT[:, csl], raws[2], j)
                for b in range(SC // 128):
                    st = j * (SC // 128) + b
                    # shares the 2-buf rope_u slots so back-to-back
                    # transposes double-buffer against their copies
                    ps_t = ps_ru.tile([128, 128], BF16, tag="rope_u",
                                      name="ps_t")
                    nc.tensor.transpose(
                        ps_t, raws[3][:, b * 128:(b + 1) * 128], ident
                    )
                    if b % 2 == 0:
                        nc.scalar.copy(out=v_sd[:, st, :], in_=ps_t)
                    else:
                        nc.vector.tensor_copy(out=v_sd[:, st, :], in_=ps_t)

            pend = None
            for j in range(NSC):
                csl = slice(j * SC, (j + 1) * SC)
                ps_ps = [ps_proj.tile([128, SC], F32, tag="mm", name=f"pp{t}")
                         for t in range(4)]
                for kc in range(NKC):
                    for t in range(4):  # q0, q1, k, v (columns of wqkv)
                        nc.tensor.matmul(
                            ps_ps[t],
                            wqkv_sb[:, kc, t * 128:(t + 1) * 128],
                            xsb[:, kc, csl],
                            start=(kc == 0),
                            stop=(kc == NKC - 1),
                        )
                raws = [p1s.tile([128, SC], BF16, tag="raw", name=f"raw{t}")
                        for t in range(4)]
                for t in range(4):
                    if t % 2 == 0:
                        nc.scalar.copy(out=raws[t], in_=ps_ps[t])
                    else:
                        nc.vector.tensor_copy(out=raws[t], in_=ps_ps[t])
                if pend is not None:
                    consume(*pend)
                pend = (j, raws)
            consume(*pend)

        # ============ phase 2: attention + output projection ===============
        with tc.tile_pool(name="ps_mm", bufs=3, space="PSUM") as ps_mm, \
             tc.tile_pool(name="ps_acc", bufs=2, space="PSUM") as ps_acc, \
             tc.tile_pool(name="ps_den", bufs=1, space="PSUM") as ps_den, \
             tc.tile_pool(name="ps_outc", bufs=2, space="PSUM") as ps_outc:

            chunk_state = {}

            def attention_chunk(j, fillers=()):
                fillers = list(fillers)
                csl = slice(j * SC, (j + 1) * SC)
                nk = 4 * j + 4  # causal: sk tiles 0..4j+3
                fill_at = set()
                if fillers:
                    step = max(1, (nk - 3) // len(fillers))
                    fill_at = {i * step for i in range(len(fillers))}
                qcs = [qTs[h][:, csl] for h in range(QH_PER_CORE)]
                ps_os = [ps_acc.tile([128, SC], F32, tag="attn",
                                     name=f"ps_o{h}")
                         for h in range(QH_PER_CORE)]
                ps_dall = ps_den.tile([64, SC], F32, tag="den", name="ps_dall")
                ps_ds = [ps_dall[32 * h:32 * h + 1, :]
                         for h in range(QH_PER_CORE)]
                # software-pipelined: scores/exp for step k are emitted
                # before attnV/den of step k-1, so the PE never waits on the
                # ACT exp latency
                pend = None  # (k, vsl, e_tiles)

                def flush(pk, pvsl, pes, last):
                    # AV pair first, then the den pair adjacent (the two den
                    # matmuls hit col groups 0/1 and run concurrently);
                    # on the last step den goes first so the recip chain
                    # starts while the final AV pair still streams
                    av = [(h, 'av') for h in range(QH_PER_CORE)]
                    dn = [(h, 'dn') for h in range(QH_PER_CORE)]
                    order = dn + av if last else av + dn
                    for h, kind in order:
                        if kind == 'av':
                            nc.tensor.matmul(
                                ps_os[h][:, pvsl], v_sd[:, pk, :],
                                pes[h][:, pvsl],
                                start=(pk == 0), stop=last,
                            )
                        else:
                            nc.tensor.matmul(
                                ps_ds[h][:, pvsl], ones_col, pes[h][:, pvsl],
                                start=(pk == 0), stop=last,
                            )

                for k in range(nk):
                    m = k - 4 * j
                    # diagonal tiles (m>=0) only touch cols >= 128m
                    v0 = max(m, 0) * 128
                    vsl = slice(v0, SC)
                    kc_t = kT[:, k * 128:(k + 1) * 128]
                    es = []
                    for h in range(QH_PER_CORE):
                        ps_s = ps_mm.tile([128, SC], F32, tag="mm",
                                          name="ps_s")
                        nc.tensor.matmul(
                            ps_s[:, vsl], kc_t, qcs[h][:, vsl],
                            start=True, stop=True,
                        )
                        e = expp.tile([128, SC], BF16, tag="exp", name="e")
                        nc.scalar.activation(
                            out=e[:, vsl], in_=ps_s[:, vsl],
                            func=mybir.ActivationFunctionType.Exp,
                            scale=SCALE,
                        )
                        if m >= 0:
                            dsl = slice(m * 128, (m + 1) * 128)
                            nc.vector.tensor_mul(
                                out=e[:, dsl], in0=e[:, dsl], in1=mask01,
                            )
                        es.append(e)
                    if pend is not None:
                        flush(*pend, last=False)
                    pend = (k, vsl, es)
                    if k in fill_at and fillers:
                        fillers.pop(0)()
                flush(*pend, last=True)
                for f in fillers:
                    f()
                chunk_state[j] = (ps_os, ps_dall)

            def norm_chunk(j):
                """den -> recip (DVE) -> PE broadcast -> normalize into aoT."""
                csl = slice(j * SC, (j + 1) * SC)
                ps_os, ps_dall = chunk_state.pop(j)
                recip = p2s.tile([64, SC], F32, tag="recip", name="recip")
                nc.vector.reciprocal_approx_fast(out=recip, in_=ps_dall)
                for h in range(QH_PER_CORE):
                    db_ps = ps_mm.tile([128, SC], F32, tag="mm",
                                       name=f"db{h}")
                    nc.tensor.matmul(
                        db_ps,
                        ones_f32[32 * h:32 * h + 1, :],
                        recip[32 * h:32 * h + 1, :],
                        start=True, stop=True,
                    )
                    db_s = p2s.tile([128, SC], F32, tag="db_s",
                                    name=f"dbs{h}")
                    nc.vector.tensor_copy(out=db_s, in_=db_ps)
                    nc.vector.tensor_mul(out=aoTs[h][:, csl],
                                         in0=ps_os[h], in1=db_s)

            def outproj_st(j, b):
                """one 128-row block of the output projection of chunk j."""
                st = j * (SC // 128) + b
                ssl = slice(st * 128, (st + 1) * 128)
                ob = outp.tile([128, HID], BF16, tag="outb", name="ob")
                for hc in range(NSC):
                    hsl = slice(hc * SC, (hc + 1) * SC)
                    ps_c = ps_outc.tile([128, SC], F32, tag="oc",
                                        name="ps_c")
                    for h in range(QH_PER_CORE):
                        nc.tensor.matmul(
                            ps_c,
                            aoTs[h][:, ssl],
                            wo_sb[:, h, hsl],
                            start=(h == 0),
                            stop=(h == QH_PER_CORE - 1),
                        )
                    if hc % 2 == 0:
                        nc.scalar.copy(out=ob[:, hsl], in_=ps_c)
                    else:
                        nc.vector.tensor_copy(out=ob[:, hsl], in_=ps_c)
                eng = nc.sync if st % 2 == 0 else nc.scalar
                eng.dma_start(outd[ssl, :], ob)

            attention_chunk(0)
            norm_chunk(0)
            for j in range(1, NSC):
                fillers = [
                    (lambda jj, bb: lambda: outproj_st(jj, bb))(j - 1, b)
                    for b in range(SC // 128)
                ]
                attention_chunk(j, fillers)
                norm_chunk(j)
            for b in range(SC // 128):
                outproj_st(NSC - 1, b)

    nc.finalize()
    return nc


def shard_inputs(x, cos, sin, wq, wk, wv, wo):
    x = np.asarray(x, np.float32).reshape(S, HID)
    cos = np.asarray(cos, np.float32)
    sin = np.asarray(sin, np.float32)
    wq = np.asarray(wq, np.float32)
    wk = np.asarray(wk, np.float32)
    wv = np.asarray(wv, np.float32)
    wo = np.asarray(wo, np.float32)

    # [NSC, 128, NKC, SC]: per (chunk, partition) a contiguous 16KB block
    xT = x.T.astype(NPBF)  # [HID, S]
    xt_fat = np.ascontiguousarray(
        xT.reshape(NKC, 128, NSC, SC).transpose(2, 1, 0, 3))

    cos_h, sin_h = cos[:, :HH].T, sin[:, :HH].T       # [64, S]
    c1 = np.ascontiguousarray(
        np.concatenate([cos_h, -sin_h], axis=0)).astype(NPBF)
    c2 = np.ascontiguousarray(
        np.concatenate([sin_h, cos_h], axis=0)).astype(NPBF)

    r1 = np.zeros((HD, HD), np.float32)
    for i in range(HH // 2):
        r1[2 * i, 2 * i + 1] = -1.0
        r1[2 * i + 1, 2 * i] = 1.0
    r1[HH:, :] = r1[:HH, :]
    r2 = np.zeros((HD, HD), np.float32)
    for d in range(HH):
        r2[d, d + HH] = 1.0
        r2[d + HH, d + HH] = 1.0
    r1t = np.ascontiguousarray(r1.T).astype(NPBF)  # lhsT for out = R1 @ rhs
    r2t = np.ascontiguousarray(r2.T).astype(NPBF)

    in_maps = []
    for c in range(NCORES):
        h0 = QH_PER_CORE * c
        kvh = h0 * NKV // NH
        wq_c = wq[h0 * HD:(h0 + QH_PER_CORE) * HD, :]    # [256, HID]
        wk_c = wk[kvh * HD:(kvh + 1) * HD, :]            # [128, HID]
        wv_c = wv[kvh * HD:(kvh + 1) * HD, :]
        wqkvT_c = np.concatenate([wq_c, wk_c, wv_c], axis=0).T.astype(NPBF)
        # [128, NKC, 512]: per partition a contiguous 16KB block
        wqkv_fat = np.ascontiguousarray(
            wqkvT_c.reshape(NKC, 128, 512).transpose(1, 0, 2))
        woT_c = np.ascontiguousarray(
            wo[:, h0 * HD:(h0 + QH_PER_CORE) * HD].T).astype(NPBF)
        in_maps.append({
            "xt": xt_fat,
            "wqkvt": wqkv_fat,
            "wot": woT_c,
            "c1": c1,
            "c2": c2,
            "r1t": r1t,
            "r2t": r2t,
        })
    return in_maps


_CACHED_NC = None


def kernel(x, cos, sin, wq, wk, wv, wo, _trace=False, _tmpdir=None):
    global _CACHED_NC
    in_maps = shard_inputs(x, cos, sin, wq, wk, wv, wo)
    if _CACHED_NC is None:
        _CACHED_NC = build_graph()
    nc = _CACHED_NC
    res = bass_utils.run_bass_kernel_spmd(
        nc, in_maps, core_ids=list(range(NCORES)),
        trace=_trace, tmpdir=_tmpdir,
    )
    total = np.zeros((S, HID), np.float32)
    for r in res.results:
        total += np.asarray(r["out"], dtype=np.float32)
    out = total.reshape(1, S, HID)
    if _trace:
        return out, res
    return out
